# revision 18
# baseline (speedup 1.0000x reference)
"""Trainium2 Bass kernel for the three-GEU (text/video/audio) embedding model.

Strategy (8 NeuronCores, one chip). Collective latency dominates this part
(the first collective also absorbs the multi-core launch skew), so the design
minimizes the number and depth of serial collectives:

  - GEMM1 is K-sharded: core c holds input-feature slice [512c, 512(c+1)) of
    every GEU input and the matching weight row slice W1[:, 512c:512c+512].T.
    All preprocessing (text max-pool, audio ragged-mean) is local to the
    core's slice -- no pre-GEMM collective at all.
  - Each core computes partial h = x_shard @ W1_shard for all three embeds,
    batch-major [64, 3*4096] fp16, staged into a core-major-blocked DRAM
    buffer so a single ReduceScatter both sums the partials and hands core c
    exactly its output-column shard h[:, 512c:512(c+1)) -- the same access
    pattern on every core (SPMD-safe, no per-core slicing).
  - GEMM2 is likewise K-sharded over h columns (weight row slices), a second
    ReduceScatter delivers g shards, then GLU + partial norms + one tiny
    AllReduce + rsqrt scaling.
  - Collective chain: RS(video h) -> RS(text+audio h) -> RS(g) -> AR(norms).
    The video RS is staged ~20us in (video needs no preprocessing and wv
    streams first), absorbing most of the launch skew while the text/audio
    GEMM1 partials still run locally.
  - PE warm-up matmuls at t=0 ramp the tensor-engine DVFS clock during the
    initial DMA fill; weights stream on the two HWDGE rings in consumption
    order; fp16 operands, fp32 PSUM accumulation.
"""

import numpy as np

B = 64
L = 30
D = 4096
DA = 1024
T = 128
NCORES = 8
S = D // NCORES     # 512: per-core shard of D
SA = DA // NCORES   # 128: per-core shard of Da
JC = 8              # output column chunks (of 512) per GEMM
NE = 3              # embeds

# contraction k-tiles per weight matrix (weight chunk = one k-tile x 4096)
WKT = {"wv": 4, "wt": 4, "wa": 1, "wgv": 4, "wgt": 4, "wga": 4}

_STATE: dict = {}


def _build():
    from contextlib import ExitStack

    import concourse.bass as bass
    import concourse.tile as tile
    from concourse import bacc, mybir
    from concourse.bass import ts
    from concourse.masks import make_identity

    fp16 = mybir.dt.float16
    f32 = mybir.dt.float32
    AX = mybir.AxisListType
    ALU = mybir.AluOpType
    ACTF = mybir.ActivationFunctionType

    nc = bacc.Bacc(
        "TRN2",
        target_bir_lowering=False,
        debug=False,
        enable_asserts=False,
        num_devices=NCORES,
    )
    RG = [list(range(NCORES))]

    # --- kernel I/O (per-core shards, staged by the host wrapper) ---
    w_in = {
        name: nc.dram_tensor(name, [nkt, 128, JC * S], fp16,
                             kind="ExternalInput")
        for name, nkt in WKT.items()
    }
    textT = nc.dram_tensor("textT", [S, B, L], fp16, kind="ExternalInput")
    audioT = nc.dram_tensor("audioT", [T, B, SA], fp16, kind="ExternalInput")
    vT_d = nc.dram_tensor("vT", [128, 4, B], fp16, kind="ExternalInput")
    maskT_d = nc.dram_tensor("maskT", [T, B], fp16, kind="ExternalInput")
    biases_d = nc.dram_tensor("biases", [1, 3 * D], fp16, kind="ExternalInput")
    b1T_d = nc.dram_tensor("b1T", [128, 3 * 4], f32, kind="ExternalInput")
    EMBEDS = ("video", "text", "audio")
    out_d = {
        e: nc.dram_tensor(f"out_{e}", [B, S], f32, kind="ExternalOutput")
        for e in EMBEDS
    }
    BIAS2 = {"video": 0, "text": 1, "audio": 2}

    with ExitStack() as ctx:
        tc = ctx.enter_context(tile.TileContext(nc))

        persist = ctx.enter_context(tc.tile_pool(name="persist", bufs=1))
        wpool = ctx.enter_context(tc.tile_pool(name="wstream", bufs=8))
        txpool = ctx.enter_context(tc.tile_pool(name="txpool", bufs=4))
        work = ctx.enter_context(tc.tile_pool(name="work", bufs=2))
        ypool = ctx.enter_context(tc.tile_pool(name="ypool", bufs=3))
        psum = ctx.enter_context(tc.tile_pool(name="psum", bufs=2, space="PSUM"))
        dram = ctx.enter_context(tc.tile_pool(name="dram", bufs=1, space="DRAM"))

        # ---- persistent SBUF tiles ----
        au_sb = persist.tile([T, B, SA], fp16)        # audio shard [t, b, c']
        vt_sb = persist.tile([128, 4, B], fp16)       # video shard k-tiles
        msk_sb = persist.tile([T, B], fp16)           # mask/nf, transposed
        bias_sb = persist.tile([1, 3, D], fp16)       # gating bias rows (/8)
        b1t_sb = persist.tile([128, 3, 4], f32)       # my GEMM1 bias slice, T
        ones_sb = persist.tile([1, B], fp16)
        ident = persist.tile([B, B], fp16)
        ident128 = persist.tile([128, 128], fp16)
        warm16 = persist.tile([128, 512], fp16)
        stg = persist.tile([128, 5, B], fp16)         # local xT: 4 text + 1 audio
        hpart = persist.tile([B, NE, D], fp16)        # GEMM1 partials
        gpart = persist.tile([B, NE, D], fp16)        # GEMM2 partials
        rs1out = persist.tile([B, NE, S], fp16)       # my h shard (post RS)
        rs2out = persist.tile([B, NE, S], fp16)       # my g shard (post RS)
        hstat = persist.tile([128, NE, 4, B], fp16)   # my h shard transposed
        hb16 = {ei: persist.tile([B, S], fp16, name=f"hb16_{ei}")
                for ei in range(NE)}                  # biased h shard, batch-major
        nsq = persist.tile([B, 4], f32)
        nsqg = persist.tile([B, 3], f32)
        nrm = persist.tile([B, 3], f32)
        rcp = persist.tile([B, 3], f32)
        hview = hpart.rearrange("b e (c s) -> b e c s", s=S)
        gview = gpart.rearrange("b e (c s) -> b e c s", s=S)

        # ---- constants ----
        nc.gpsimd.memset(ones_sb[:], 1.0)
        nc.vector.memset(nsq[:], 0.0)
        nc.vector.memset(warm16[:], 0.0)
        make_identity(nc, ident[:])
        make_identity(nc, ident128[:])

        # ---- input DMAs: all inputs ahead of any weight chunk ----
        nc.sync.dma_start(au_sb[:], audioT.ap())
        nc.sync.dma_start(msk_sb[:], maskT_d.ap())
        nc.scalar.dma_start(vt_sb[:], vT_d.ap())
        nc.scalar.dma_start(bias_sb[0:1, :, :], biases_d.ap())
        nc.scalar.dma_start(b1t_sb.rearrange("p e t -> p (e t)"), b1T_d.ap())
        t_view = textT.ap().rearrange("(n p) b l -> n p b l", p=128)
        tx = []
        for i in range(4):
            t = txpool.tile([128, B, L], fp16, name=f"tx{i}", tag="tx")
            [nc.scalar, nc.sync][i % 2].dma_start(t[:], t_view[i])
            tx.append(t)

        # ---- weight chunk DMAs: issued in consumption order on alternating
        # rings. The wgt/wga issues are deferred until after the GEMM1 drain
        # instructions are queued, so a pool-buffer wait on an RS-gated GEMM2
        # can never sit AHEAD of the drains that feed the first RS (deadlock).
        hwdge = [nc.sync, nc.scalar]
        wtiles = {name: [] for name in WKT}
        qi = [0]

        def queue_weights(names):
            for name in names:
                for kt in range(WKT[name]):
                    w = wpool.tile([128, JC, S], fp16, name=f"{name}{kt}",
                                   tag="wchunk")
                    hwdge[qi[0] % 2].dma_start(
                        w[:],
                        w_in[name].ap()[kt].rearrange("p (a n) -> p a n", n=S))
                    qi[0] += 1
                    wtiles[name].append(w)

        queue_weights(("wv", "wt", "wa", "wgv"))

        # PSUM layout: g_ps tag 2 bufs x 3 banks + hT_ps 1 + aT 1 -> 8 banks.
        def gemm_ps(name):
            return psum.tile([B, 3, S], f32, name=name, tag="g_ps")

        # ---- PE warm-up: ramp the DVFS clock during the DMA fill.
        # Many short matmuls keep the PE continuously busy ~4us (the DVFS
        # ramp needs ~3us of uninterrupted execution) yet drain instantly
        # when real work arrives.
        warm_ps = gemm_ps("warm")
        for _ in range(10):
            nc.tensor.matmul(warm_ps[:, 0, :], ident[:], warm16[0:B, :],
                             start=True, stop=True, skip_group_check=True)
        for _ in range(40):
            nc.tensor.matmul(warm_ps[:, 0, 0:128], ident[:],
                             warm16[0:B, 0:128],
                             start=True, stop=True, skip_group_check=True)

        # ---- text max-pool over L -> stg[:, 0:4, :] (DVE) ----
        for i in range(4):
            nc.vector.reduce_max(stg[:, i, :], tx[i][:], AX.X)

        # ---- audio ragged masked-mean: 64 PE matvecs -> stg[:, 4, :] ----
        aT_ps = psum.tile([SA, B], f32, name="aT", tag="aT", bufs=1)
        for b in range(B):
            nc.tensor.matmul(
                aT_ps[:, b:b + 1], au_sb[:, b, :], msk_sb[:, b:b + 1],
                start=True, stop=True)
        nc.vector.tensor_copy(stg[:, 4, :], aT_ps[:])

        # local stationary k-tiles [128, B]
        LHS = {
            "video": lambda k: vt_sb[:, k, :],
            "text": lambda k: stg[:, k, :],
            "audio": lambda k: stg[:, 4, :],
        }

        drain_eng = [0]

        def kgemm(e, stage, wname, pview):
            """Partial GEMM: pview[:, ei, jc, :] = bias + x_loc @ W_slice.

            Column chunks in 3 passes of <=3 (double-buffered PSUM), k inner.
            GEMM1 drains run on DVE ONLY: the scalar sequencer can be blocked
            at a late weight-chunk dma_start whose pool buffer waits on an
            RS-gated GEMM2 -- a scalar drain queued behind it would deadlock
            the RS staging chain. By GEMM2 time the scalar ring is past all
            weight DMAs, so stage-2 drains may alternate DVE/scalar.
            """
            ei = EMBEDS.index(e)
            nkt = WKT[wname]
            lhs = LHS[e] if stage == 1 else (
                lambda k, ei=ei: hstat[:, ei, k, :])
            for p0 in range(0, JC, 3):
                pw = min(3, JC - p0)
                ps = gemm_ps(f"{wname}_p{p0}")
                for j in range(pw):
                    jc = p0 + j
                    if stage == 2:
                        nc.tensor.matmul(
                            ps[:, j, :], ones_sb[:],
                            bias_sb[:, BIAS2[e], ts(jc, S)],
                            start=True, stop=False)
                    for k in range(nkt):
                        nc.tensor.matmul(
                            ps[:, j, :], lhs(k), wtiles[wname][k][:, jc, :],
                            start=(stage == 1 and k == 0),
                            stop=(k == nkt - 1))
                use_scalar = drain_eng[0] % 2 == 1
                drain_eng[0] += 1
                if use_scalar:
                    nc.scalar.copy(pview[:, ei, p0:p0 + pw, :],
                                   ps[:, 0:pw, :])
                else:
                    nc.vector.tensor_copy(pview[:, ei, p0:p0 + pw, :],
                                          ps[:, 0:pw, :])

        # ---- ReduceScatter: core-major-blocked, batch-major payload ----
        def launch_rs(name, src, dst, e0, n):
            rs_in = dram.tile([NCORES * B, n * S], fp16, name=f"{name}_in")
            rs_out = dram.tile([B, n * S], fp16, name=f"{name}_out")
            # chunk c = rows [64c, 64c+64): partials for core c's columns
            iv = rs_in.rearrange("(c b) (e j) -> b e c j", c=NCORES, e=n)
            nc.gpsimd.dma_start(
                iv, src[:, e0:e0 + n, :].rearrange("b e (c j) -> b e c j",
                                                   c=NCORES))
            nc.gpsimd.collective_compute(
                "ReduceScatter", ALU.add, replica_groups=RG,
                ins=[rs_in.opt()], outs=[rs_out.opt()])
            nc.gpsimd.dma_start(
                dst[:, e0:e0 + n, :],
                rs_out.rearrange("b (e j) -> b e j", e=n))

        # ================= main schedule =================
        kgemm("video", 1, "wv", hview)
        launch_rs("rsv", hpart, rs1out, 0, 1)      # early skew sink

        kgemm("text", 1, "wt", hview)
        queue_weights(("wgt",))
        kgemm("audio", 1, "wa", hview)
        queue_weights(("wga",))
        launch_rs("rsta", hpart, rs1out, 1, 2)

        def transpose_shard(ei):
            """rs1out[:, ei] -> hstat (T-layout), add GEMM1 bias there, and
            back-transpose the biased shard to hb16 for the GLU product."""
            hT_ps = psum.tile([128, 4, B], fp16, name=f"hT_{ei}", tag="hT_ps",
                              bufs=1)
            for j in range(4):
                nc.tensor.transpose(hT_ps[:, j, :],
                                    rs1out[:, ei, ts(j, 128)], ident[:])
            nc.vector.tensor_copy(hstat[:, ei, :, :], hT_ps[:])
            for t in range(4):
                nc.vector.tensor_scalar_add(hstat[:, ei, t, :],
                                            hstat[:, ei, t, :],
                                            b1t_sb[:, ei, t:t + 1])
            hb_ps = psum.tile([B, 4, 128], fp16, name=f"hb_{ei}",
                              tag="hT_ps", bufs=1)
            for t in range(4):
                nc.tensor.transpose(hb_ps[:, t, :], hstat[:, ei, t, :],
                                    ident128[:])
            nc.vector.tensor_copy(
                hb16[ei].rearrange("b (t j) -> b t j", t=4), hb_ps[:])

        transpose_shard(0)
        kgemm("video", 2, "wgv", gview)
        transpose_shard(1)
        transpose_shard(2)
        kgemm("text", 2, "wgt", gview)
        kgemm("audio", 2, "wga", gview)
        launch_rs("rsg", gpart, rs2out, 0, 3)

        # ---- GLU + partial norms ----
        y_sb = []
        for ei, e in enumerate(EMBEDS):
            sg16 = work.tile([B, S], fp16, name=f"sg_{e}", tag="sg16")
            nc.scalar.activation(sg16[:], rs2out[:, ei, :], ACTF.Sigmoid)
            y = ypool.tile([B, S], f32, name=f"y_{e}", tag="y")
            nc.vector.tensor_mul(y[:], hb16[ei][:], sg16[:])
            ysq = work.tile([B, S], f32, name=f"ysq_{e}", tag="ysq")
            nc.vector.tensor_mul(ysq[:], y[:], y[:])
            nc.vector.reduce_sum(nsq[:, ei:ei + 1], ysq[:], AX.X)
            y_sb.append(y)

        # ---- AllGather norm partials + local sum; normalize; write out ----
        ag_in = dram.tile([B, 3], f32)
        ag_out = dram.tile([NCORES * B, 3], f32, addr_space="Shared")
        nc.gpsimd.dma_start(ag_in[:], nsq[:, 0:3])
        nc.gpsimd.collective_compute(
            "AllGather", ALU.bypass, replica_groups=RG,
            ins=[ag_in.opt()], outs=[ag_out.opt()])
        nsq8 = persist.tile([B, 3, NCORES], f32)
        nc.gpsimd.dma_start(nsq8[:],
                            ag_out.rearrange("(r p) e -> p e r", p=B))
        nc.vector.reduce_sum(nsqg[:], nsq8[:], AX.X)
        nc.scalar.sqrt(nrm[:], nsqg[:])
        nc.vector.tensor_scalar_max(nrm[:], nrm[:], 1e-12)
        nc.vector.reciprocal(rcp[:], nrm[:])
        for ei, e in enumerate(EMBEDS):
            yo = work.tile([B, S], f32, name=f"yo_{e}", tag="ysq")
            nc.vector.tensor_scalar_mul(yo[:], y_sb[ei][:],
                                        rcp[:, ei:ei + 1])
            nc.sync.dma_start(out_d[e].ap(), yo[:])

    nc.compile()
    return nc


def _get_nc():
    if "nc" not in _STATE:
        _STATE["nc"] = _build()
    return _STATE["nc"]


def _prep_inputs(text, video, audio_feats, Wt, bt, Wgt, bgt, Wv, bv, Wgv, bgv,
                 Wa, ba, Wga, bga, nframes, raw_audio_len):
    """Shard + transpose + fp16-cast the full inputs into per-core in_maps."""
    f16 = np.float16
    text = np.asarray(text, dtype=np.float32)
    video = np.asarray(video, dtype=np.float32)
    audio = np.asarray(audio_feats, dtype=np.float32)

    ratio = int(round(float(np.asarray(raw_audio_len)) / T))
    nf = np.maximum(
        1, (np.asarray(nframes).astype(np.float32) / ratio).astype(np.int32))
    mask = (np.arange(T)[None, :] < nf[:, None]).astype(np.float32)
    mask = mask / nf[:, None].astype(np.float32)          # [B, T] mask/nf
    maskT = np.ascontiguousarray(mask.T).astype(f16)      # [T, B]

    def wrow(W, sl, nkt):
        """W[:, sl].T [512|128, 4096] -> [nkt, 128, JC*S] k-tile chunks."""
        wt = np.ascontiguousarray(W[:, sl].T)             # [K_shard, 4096]
        return np.ascontiguousarray(
            wt.reshape(nkt, 128, 4096)).astype(f16)

    # gating biases ride the GEMM2 partials, which are summed across the 8
    # cores by the ReduceScatter -> pre-scale by 1/8. GEMM1 biases are added
    # AFTER the ReduceScatter (full h) in transposed layout -> full scale.
    bias_rows = np.stack([
        np.asarray(b, dtype=np.float32) / NCORES for b in (bgv, bgt, bga)
    ]).reshape(1, -1).astype(f16)
    b1 = np.stack([np.asarray(b, dtype=np.float32) for b in (bv, bt, ba)])

    in_maps = []
    for c in range(NCORES):
        sl = slice(c * S, (c + 1) * S)
        sla = slice(c * SA, (c + 1) * SA)
        vT = np.ascontiguousarray(
            video[:, sl].T.reshape(4, 128, B).transpose(1, 0, 2)).astype(f16)
        m = {
            "wv": wrow(Wv, sl, 4),
            "wt": wrow(Wt, sl, 4),
            "wa": wrow(Wa, sla, 1),
            "wgv": wrow(Wgv, sl, 4),
            "wgt": wrow(Wgt, sl, 4),
            "wga": wrow(Wga, sl, 4),
            "textT": np.ascontiguousarray(
                text[:, :, sl].transpose(2, 0, 1)).astype(f16),
            "audioT": np.ascontiguousarray(
                audio[:, sla, :].transpose(2, 0, 1)).astype(f16),
            "vT": vT,
            "maskT": maskT,
            "biases": bias_rows,
            "b1T": np.ascontiguousarray(
                b1[:, sl].reshape(3, 4, 128).transpose(2, 0, 1)
                .reshape(128, 12)).astype(np.float32),
        }
        in_maps.append(m)
    return in_maps


def kernel(text, video, audio_feats, Wt, bt, Wgt, bgt, Wv, bv, Wgv, bgv,
           Wa, ba, Wga, bga, nframes, raw_audio_len):
    from concourse.bass_utils import run_bass_kernel_spmd

    nc = _get_nc()
    in_maps = _prep_inputs(text, video, audio_feats, Wt, bt, Wgt, bgt,
                           Wv, bv, Wgv, bgv, Wa, ba, Wga, bga,
                           nframes, raw_audio_len)
    res = run_bass_kernel_spmd(nc, in_maps, list(range(NCORES)))
    _STATE["last_results"] = res
    outs = []
    for e in ("text", "video", "audio"):
        outs.append(np.concatenate(
            [res.results[c][f"out_{e}"] for c in range(NCORES)], axis=1))
    return tuple(outs)
